# revision 16
# baseline (speedup 1.0000x reference)
"""Trainium2 Bass kernel for nn_AttentionBlock (GroupNorm + MHA + proj + residual).

Sharding: data-parallel over batch — 8 batch elements, one per NeuronCore.
Each core runs the full block for its batch element; no collectives.

Per-core dataflow (c=512, n=1024, heads=8, d=64, groups=32):
  - GroupNorm: per-channel bn_stats/bn_aggr (DVE), group aggregation via a tiny
    f32 matmul with a 1/16 selector matrix, broadcast back to channels via a
    second tiny matmul (PE), normalize fused into one DVE pass producing bf16 y.
  - qkv as matmuls against host-pre-transposed bf16 weights. q/k produced in
    [row, n] layout; v produced directly transposed ([n, vrow]) by swapping
    matmul operands, with a ones-column appended per head (vplus) so the
    attention*V matmul also produces the softmax denominator Z exactly (fp32
    PSUM accumulation).
  - S^T = k_h^T q_h per head in [m, n] layout (K=64 matmuls), exp on ScalarE
    straight out of PSUM into bf16 SBUF.
  - out_h = [v_h^T | 1]^T @ expS^T accumulated over m-tiles -> rows 0..63 are
    the unnormalized output, row 64 is Z. Normalize with reciprocal +
    partition_broadcast + one DVE multiply.
  - proj matmul, then (P + pb_eff) + x fused in one DVE pass.

Host-side algebraic folds (exact):
  - attention scale folded into q weights/bias
  - k bias dropped (row-constant shift is softmax-invariant)
  - v bias folded into proj bias: pb_eff = proj_b + proj_w @ v_b
"""

import sys

for _p in ("/opt/trn_rl_repo", "/root/.axon_site/_ro/trn_rl_repo"):
    if _p not in sys.path:
        sys.path.insert(0, _p)

from contextlib import ExitStack

import ml_dtypes
import numpy as np

import concourse.bass as bass
import concourse.bacc as bacc
import concourse.tile as tile
from concourse import mybir
from concourse.bass_utils import run_bass_kernel_spmd

F32 = mybir.dt.float32
BF16 = mybir.dt.bfloat16
AF = mybir.ActivationFunctionType
OP = mybir.AluOpType

B = 8
C = 512
N = 1024
HEADS = 8
D = 64
GROUPS = 32
GSIZE = C // GROUPS  # 16 channels per group
CT = C // 128  # 4 channel tiles
NT = N // 128  # 8 spatial tiles
W3 = 3 * C
EPS = 1e-5
NCORES = 8


def _build(nc: bass.Bass):
    x = nc.declare_dram_parameter("x", [C, N], F32, isOutput=False)
    qkvwT = nc.declare_dram_parameter("qkvwT", [C, W3], BF16, isOutput=False)
    projwT = nc.declare_dram_parameter("projwT", [C, C], BF16, isOutput=False)
    qb = nc.declare_dram_parameter("qb", [C], F32, isOutput=False)
    pbeff = nc.declare_dram_parameter("pbeff", [C], F32, isOutput=False)
    nw = nc.declare_dram_parameter("nw", [C], F32, isOutput=False)
    nb = nc.declare_dram_parameter("nb", [C], F32, isOutput=False)
    sel = nc.declare_dram_parameter("sel", [CT, 128, GROUPS], F32, isOutput=False)
    selb = nc.declare_dram_parameter("selb", [CT, GROUPS, 128], F32, isOutput=False)
    out = nc.declare_dram_parameter("out", [C, N], F32, isOutput=True)

    with tile.TileContext(nc) as tc, ExitStack() as ctx:
        singles = ctx.enter_context(tc.tile_pool(name="singles", bufs=1))
        small = ctx.enter_context(tc.tile_pool(name="small", bufs=4))
        work = ctx.enter_context(tc.tile_pool(name="work", bufs=2))
        expp = ctx.enter_context(tc.tile_pool(name="expp", bufs=4))
        gn_ctx = ExitStack()
        gnps = gn_ctx.enter_context(tc.tile_pool(name="gnps", bufs=5, space="PSUM"))

        x_sb = singles.tile([128, CT * N], F32)
        y_sb = singles.tile([128, CT * N], BF16)
        q_sb = singles.tile([128, 4 * N], BF16)
        k_sb = singles.tile([128, 4 * N], BF16)
        vplus = singles.tile([128, NT * HEADS * (D + 1)], BF16)  # [nt][h][65]
        av_sb = singles.tile([128, CT * N], BF16)
        wqkv_sb = singles.tile([128, CT * W3], BF16)
        wproj_sb = singles.tile([128, CT * C], BF16)
        bias_sb = singles.tile([128, 16], F32)  # 0:4 qb | 4:8 pbeff | 8:12 nw | 12:16 nb
        sel_sb = singles.tile([128, CT * GROUPS], F32)
        selb_sb = singles.tile([GROUPS, CT * 128], F32)
        zero_sb = singles.tile([128, 1], F32)
        eps_sb = singles.tile([128, 1], F32)
        ab_sb = singles.tile([128, 2 * CT], F32)  # a cols 0..3, b2 cols 4..7

        nc.vector.memset(zero_sb, 0.0)
        nc.vector.memset(eps_sb, EPS)
        nc.vector.memset(vplus, 1.0)

        for t in range(CT):
            cs = slice(t * 128, (t + 1) * 128)
            nc.sync.dma_start(out=x_sb[:, t * N:(t + 1) * N], in_=x[cs, :])
            nc.sync.dma_start(out=wqkv_sb[:, t * W3:(t + 1) * W3], in_=qkvwT[cs, :])
            nc.sync.dma_start(out=wproj_sb[:, t * C:(t + 1) * C], in_=projwT[cs, :])

        nc.sync.dma_start(
            out=sel_sb[:].rearrange("p (t g) -> p t g", g=GROUPS),
            in_=sel[:].rearrange("t p g -> p t g"),
        )
        nc.sync.dma_start(
            out=selb_sb[:].rearrange("g (t p) -> g t p", p=128),
            in_=selb[:].rearrange("t g p -> g t p"),
        )
        nc.sync.dma_start(out=bias_sb[:, 0:4], in_=qb[:].rearrange("(t p) -> p t", p=128))
        nc.sync.dma_start(out=bias_sb[:, 4:8], in_=pbeff[:].rearrange("(t p) -> p t", p=128))
        nc.sync.dma_start(out=bias_sb[:, 8:12], in_=nw[:].rearrange("(t p) -> p t", p=128))
        nc.sync.dma_start(out=bias_sb[:, 12:16], in_=nb[:].rearrange("(t p) -> p t", p=128))
        # Each TPB instruction has a single HW wait slot. Absorb the four bias
        # DMA semaphores onto the DVE clock early with tiny touch copies so
        # downstream DVE ops carry at most one (PSUM) wait.
        for j in range(4):
            bt = small.tile([1, 1], F32, tag="btouch", name=f"btouch{j}")
            nc.vector.tensor_copy(out=bt, in_=bias_sb[0:1, 4 * j:4 * j + 1])

        # bf16 staging copies of the selector matrices (entries are exact in
        # bf16); also collapses downstream matmul waits onto the DVE semaphore.
        selbf = singles.tile([128, CT * GROUPS], BF16)
        selbbf = singles.tile([GROUPS, CT * 128], BF16)
        nc.vector.tensor_copy(out=selbf, in_=sel_sb)
        nc.vector.tensor_copy(out=selbbf, in_=selb_sb)

        # ---------------- GroupNorm ----------------
        # Group aggregation uses hi/lo-split bf16 matmuls (exact selector,
        # f32 PSUM accumulation) to recover ~fp32 precision without the
        # fp32-matmul wait-slot limit.
        gps = gnps.tile([GROUPS, 2], F32, tag="gn")  # [E[x], E[x^2]] per group
        for t in range(CT):
            xt = x_sb[:, t * N:(t + 1) * N]
            st = small.tile([128, 2, 6], F32, tag="bn")
            nc.vector.bn_stats(out=st[:, 0, :], in_=xt[:, 0:512])
            nc.vector.bn_stats(out=st[:, 1, :], in_=xt[:, 512:1024])
            mv = small.tile([128, 2], F32, tag="mv")
            nc.vector.bn_aggr(out=mv, in_=st)
            mv2 = small.tile([128, 2], F32, tag="mv2")  # [mean, mean^2 + var]
            nc.vector.tensor_copy(out=mv2[:, 0:1], in_=mv[:, 0:1])
            nc.vector.tensor_scalar(
                out=mv2[:, 1:2], in0=mv[:, 0:1], scalar1=mv[:, 0:1],
                scalar2=mv[:, 1:2], op0=OP.mult, op1=OP.add,
            )
            mv2hi = small.tile([128, 2], BF16, tag="mv2hi")
            nc.vector.tensor_copy(out=mv2hi, in_=mv2)
            mv2lo = small.tile([128, 2], BF16, tag="mv2lo")
            nc.vector.tensor_tensor(out=mv2lo, in0=mv2, in1=mv2hi, op=OP.subtract)
            nc.tensor.matmul(
                gps, lhsT=selbf[:, t * GROUPS:(t + 1) * GROUPS], rhs=mv2hi,
                start=(t == 0), stop=False,
            )
            nc.tensor.matmul(
                gps, lhsT=selbf[:, t * GROUPS:(t + 1) * GROUPS], rhs=mv2lo,
                start=False, stop=(t == CT - 1),
            )
        m2g = small.tile([GROUPS, 1], F32, tag="m2g")
        nc.vector.tensor_scalar(
            out=m2g, in0=gps[:, 0:1], scalar1=gps[:, 0:1], scalar2=None, op0=OP.mult
        )
        vvar = small.tile([GROUPS, 1], F32, tag="vvar")
        nc.vector.tensor_tensor(out=vvar, in0=gps[:, 1:2], in1=m2g, op=OP.subtract)
        sq = small.tile([GROUPS, 1], F32, tag="sq")
        nc.scalar.activation(out=sq, in_=vvar, func=AF.Sqrt, bias=eps_sb[0:GROUPS], scale=1.0)
        gst = small.tile([GROUPS, 2], F32, tag="gst")  # [M, rstd]
        nc.vector.tensor_copy(out=gst[:, 0:1], in_=gps[:, 0:1])
        nc.vector.reciprocal(out=gst[:, 1:2], in_=sq)
        gsthi = small.tile([GROUPS, 2], BF16, tag="gsthi")
        nc.vector.tensor_copy(out=gsthi, in_=gst)
        gstlo = small.tile([GROUPS, 2], BF16, tag="gstlo")
        nc.vector.tensor_tensor(out=gstlo, in0=gst, in1=gsthi, op=OP.subtract)
        # PE toucher: absorb the DVE tick of gsthi/gstlo onto the PE clock so
        # the gbc matmuls carry a single wait (one HW wait slot per inst).
        nc.tensor.ldweights(weights=gstlo[0:1, 0:1])

        for t in range(CT):
            gbc = gnps.tile([128, 2], F32, tag="gn", name=f"gbc{t}")
            nc.tensor.matmul(
                gbc, lhsT=selbbf[0:GROUPS, t * 128:(t + 1) * 128], rhs=gsthi,
                start=True, stop=False,
            )
            nc.tensor.matmul(
                gbc, lhsT=selbbf[0:GROUPS, t * 128:(t + 1) * 128], rhs=gstlo,
                start=False, stop=True,
            )
            at = ab_sb[:, t:t + 1]
            b2t = ab_sb[:, CT + t:CT + t + 1]
            nc.vector.tensor_scalar(
                out=at, in0=bias_sb[:, 8 + t:9 + t], scalar1=gbc[:, 1:2],
                scalar2=None, op0=OP.mult,
            )
            mtmp = small.tile([128, 1], F32, tag="mtmp")
            nc.vector.tensor_scalar(
                out=mtmp, in0=at, scalar1=gbc[:, 0:1], scalar2=None, op0=OP.mult
            )
            nc.vector.tensor_tensor(
                out=b2t, in0=bias_sb[:, 12 + t:13 + t], in1=mtmp, op=OP.subtract
            )
            nc.vector.tensor_scalar(
                out=y_sb[:, t * N:(t + 1) * N], in0=x_sb[:, t * N:(t + 1) * N],
                scalar1=at, scalar2=b2t, op0=OP.mult, op1=OP.add,
            )

        gn_ctx.close()
        ps = ctx.enter_context(tc.tile_pool(name="ps", bufs=2, space="PSUM"))
        psav = ctx.enter_context(tc.tile_pool(name="psav", bufs=2, space="PSUM"))

        # ---------------- QKV ----------------
        # PE touchers: absorb the weight-DMA semaphores before the matmuls.
        for kt in range(CT):
            nc.tensor.ldweights(weights=wqkv_sb[0:1, kt * W3:kt * W3 + 1])
        # q/k in [row, n] layout: row-tiles 0..3 -> q, 4..7 -> k
        for mt in range(8):
            pp = ps.tile([128, N], F32, tag="ps")
            for nh in range(2):
                for kt in range(CT):
                    nc.tensor.matmul(
                        pp[:, nh * 512:(nh + 1) * 512],
                        lhsT=wqkv_sb[:, kt * W3 + mt * 128:kt * W3 + (mt + 1) * 128],
                        rhs=y_sb[:, kt * N + nh * 512:kt * N + (nh + 1) * 512],
                        start=(kt == 0), stop=(kt == CT - 1),
                    )
            if mt < 4:
                nc.vector.tensor_scalar(
                    out=q_sb[:, mt * N:(mt + 1) * N], in0=pp,
                    scalar1=bias_sb[:, mt:mt + 1], scalar2=None, op0=OP.add,
                )
            else:
                km = mt - 4
                nc.vector.tensor_copy(out=k_sb[:, km * N:(km + 1) * N], in_=pp)
        # v directly transposed: [n, vrow], interleaved with ones column per head
        for nt in range(NT):
            vp = ps.tile([128, 512], F32, tag="ps")
            for kt in range(CT):
                nc.tensor.matmul(
                    vp,
                    lhsT=y_sb[:, kt * N + nt * 128:kt * N + nt * 128 + 128],
                    rhs=wqkv_sb[:, kt * W3 + 2 * C:kt * W3 + 3 * C],
                    start=(kt == 0), stop=(kt == CT - 1),
                )
            dst = vplus[:, nt * HEADS * (D + 1):(nt + 1) * HEADS * (D + 1)]
            dst = dst.rearrange("p (h e) -> p h e", e=D + 1)[:, :, 0:D]
            nc.vector.tensor_copy(out=dst, in_=vp.rearrange("p (h e) -> p h e", e=D))

        # ---------------- Attention (per head pair) ----------------
        for pr in range(4):
            heads = ((2 * pr, 0), (2 * pr + 1, 64))
            etiles = {}
            for h, base in heads:
                etiles[h] = expp.tile([128, NT * N], BF16, tag="exp", name=f"exp{h}")
            for mt in range(NT):
                for h, base in heads:
                    sp = ps.tile([128, N], F32, tag="ps")
                    for nh in range(2):
                        nc.tensor.matmul(
                            sp[:, nh * 512:(nh + 1) * 512],
                            lhsT=k_sb[base:base + 64, pr * N + mt * 128:pr * N + mt * 128 + 128],
                            rhs=q_sb[base:base + 64, pr * N + nh * 512:pr * N + nh * 512 + 512],
                            start=True, stop=True,
                            tile_position=(base, 0),
                        )
                    nc.scalar.activation(
                        out=etiles[h][:, mt * N:(mt + 1) * N], in_=sp,
                        func=AF.Exp, bias=zero_sb, scale=1.0,
                    )
            if pr > 0:
                # Absorb the DVE tick of the previous pair's normalize ops so
                # the av matmuls (which reuse their PSUM slots) wait only on
                # the exp (ACT) semaphore.
                nc.tensor.ldweights(weights=av_sb[0:1, (pr - 1) * N:(pr - 1) * N + 1])
            for h, base in heads:
                apn = psav.tile([D + 1, N], F32, tag="av")
                for mt in range(NT):
                    for nh in range(2):
                        nc.tensor.matmul(
                            apn[:, nh * 512:(nh + 1) * 512],
                            lhsT=vplus[:, mt * HEADS * (D + 1) + h * (D + 1):
                                       mt * HEADS * (D + 1) + (h + 1) * (D + 1)],
                            rhs=etiles[h][:, mt * N + nh * 512:mt * N + nh * 512 + 512],
                            start=(mt == 0), stop=(mt == NT - 1),
                        )
                rz = small.tile([1, N], F32, tag="rz")
                nc.vector.reciprocal(out=rz, in_=apn[D:D + 1, :])
                rzb = work.tile([D, N], F32, tag="rzb")
                nc.gpsimd.partition_broadcast(out_ap=rzb, in_ap=rz)
                nc.vector.tensor_tensor(
                    out=av_sb[base:base + 64, pr * N:(pr + 1) * N],
                    in0=apn[0:D, :], in1=rzb, op=OP.mult,
                )

        # ---------------- Proj + residual ----------------
        for kt in range(CT):
            nc.tensor.ldweights(weights=wproj_sb[0:1, kt * C:kt * C + 1])
        for ct in range(CT):
            pp = ps.tile([128, N], F32, tag="ps")
            for nh in range(2):
                for kt in range(CT):
                    nc.tensor.matmul(
                        pp[:, nh * 512:(nh + 1) * 512],
                        lhsT=wproj_sb[:, kt * C + ct * 128:kt * C + (ct + 1) * 128],
                        rhs=av_sb[:, kt * N + nh * 512:kt * N + nh * 512 + 512],
                        start=(kt == 0), stop=(kt == CT - 1),
                    )
            ob = work.tile([128, N], F32, tag="osb")
            nc.vector.scalar_tensor_tensor(
                out=ob, in0=pp, scalar=bias_sb[:, 4 + ct:5 + ct],
                in1=x_sb[:, ct * N:(ct + 1) * N], op0=OP.add, op1=OP.add,
            )
            nc.sync.dma_start(out=out[ct * 128:(ct + 1) * 128, :], in_=ob)

    return nc


_CACHE = {}


def _get_nc():
    if "nc" not in _CACHE:
        nc = bacc.Bacc()
        _build(nc)
        nc.finalize()
        _CACHE["nc"] = nc
    return _CACHE["nc"]


def prepare_in_maps(x, norm_w, norm_b, qkv_w, qkv_b, proj_w, proj_b):
    x = np.asarray(x, np.float32)
    norm_w = np.asarray(norm_w, np.float32)
    norm_b = np.asarray(norm_b, np.float32)
    qkv_w = np.asarray(qkv_w, np.float32).copy()
    qkv_b = np.asarray(qkv_b, np.float32).copy()
    proj_w = np.asarray(proj_w, np.float32)
    proj_b = np.asarray(proj_b, np.float32)

    scale = D ** -0.5
    qkv_w[:C] *= scale
    qbias = (qkv_b[:C] * scale).astype(np.float32)
    vbias = qkv_b[2 * C:3 * C]
    qkvwT = np.ascontiguousarray(qkv_w.T).astype(ml_dtypes.bfloat16)
    projwT = np.ascontiguousarray(proj_w.T).astype(ml_dtypes.bfloat16)
    pb_eff = (proj_b + proj_w @ vbias).astype(np.float32)

    sel = np.zeros([CT, 128, GROUPS], np.float32)
    selb = np.zeros([CT, GROUPS, 128], np.float32)
    for t in range(CT):
        for p in range(128):
            g = (t * 128 + p) // GSIZE
            sel[t, p, g] = 1.0 / GSIZE
            selb[t, g, p] = 1.0
    shared = dict(
        qkvwT=qkvwT, projwT=projwT, qb=qbias, pbeff=pb_eff,
        nw=norm_w, nb=norm_b, sel=sel, selb=selb,
    )
    return [
        dict(x=np.ascontiguousarray(x[i].reshape(C, N)), **shared)
        for i in range(x.shape[0])
    ]


def run(in_maps, trace=False, **kwargs):
    return run_bass_kernel_spmd(
        _get_nc(), in_maps, core_ids=list(range(NCORES)), trace=trace, **kwargs
    )


def kernel(x, norm_w, norm_b, qkv_w, qkv_b, proj_w, proj_b):
    in_maps = prepare_in_maps(x, norm_w, norm_b, qkv_w, qkv_b, proj_w, proj_b)
    res = run(in_maps)
    b, c, h, w = np.asarray(x).shape
    return np.stack(
        [res.results[i]["out"].reshape(c, h, w) for i in range(b)]
    ).astype(np.float32)


# revision 19
# speedup vs baseline: 1.1273x; 1.1273x over previous
"""Trainium2 Bass kernel for nn_AttentionBlock (GroupNorm + MHA + proj + residual).

Sharding: data-parallel over batch — 8 batch elements, one per NeuronCore.
Each core runs the full block for its batch element; no collectives.

Per-core dataflow (c=512, n=1024, heads=8, d=64, groups=32):
  - GroupNorm: per-channel bn_stats/bn_aggr (DVE), group aggregation via a tiny
    f32 matmul with a 1/16 selector matrix, broadcast back to channels via a
    second tiny matmul (PE), normalize fused into one DVE pass producing bf16 y.
  - qkv as matmuls against host-pre-transposed bf16 weights. q/k produced in
    [row, n] layout; v produced directly transposed ([n, vrow]) by swapping
    matmul operands, with a ones-column appended per head (vplus) so the
    attention*V matmul also produces the softmax denominator Z exactly (fp32
    PSUM accumulation).
  - S^T = k_h^T q_h per head in [m, n] layout (K=64 matmuls), exp on ScalarE
    straight out of PSUM into bf16 SBUF.
  - out_h = [v_h^T | 1]^T @ expS^T accumulated over m-tiles -> rows 0..63 are
    the unnormalized output, row 64 is Z. Normalize with reciprocal +
    partition_broadcast + one DVE multiply.
  - proj matmul, then (P + pb_eff) + x fused in one DVE pass.

Host-side algebraic folds (exact):
  - attention scale folded into q weights/bias
  - k bias dropped (row-constant shift is softmax-invariant)
  - v bias folded into proj bias: pb_eff = proj_b + proj_w @ v_b
"""

import sys

for _p in ("/opt/trn_rl_repo", "/root/.axon_site/_ro/trn_rl_repo"):
    if _p not in sys.path:
        sys.path.insert(0, _p)

from contextlib import ExitStack

import ml_dtypes
import numpy as np

import concourse.bass as bass
import concourse.bacc as bacc
import concourse.tile as tile
from concourse import mybir
from concourse.bass_utils import run_bass_kernel_spmd

F32 = mybir.dt.float32
BF16 = mybir.dt.bfloat16
AF = mybir.ActivationFunctionType
OP = mybir.AluOpType

B = 8
C = 512
N = 1024
HEADS = 8
D = 64
GROUPS = 32
GSIZE = C // GROUPS  # 16 channels per group
CT = C // 128  # 4 channel tiles
NT = N // 128  # 8 spatial tiles
W3 = 3 * C
EPS = 1e-5
NCORES = 8


def _build(nc: bass.Bass):
    x = nc.declare_dram_parameter("x", [C, N], F32, isOutput=False)
    qkvwT = nc.declare_dram_parameter("qkvwT", [C, W3], BF16, isOutput=False)
    projwT = nc.declare_dram_parameter("projwT", [C, C], BF16, isOutput=False)
    qb = nc.declare_dram_parameter("qb", [C], F32, isOutput=False)
    pbeff = nc.declare_dram_parameter("pbeff", [C], F32, isOutput=False)
    nw = nc.declare_dram_parameter("nw", [C], F32, isOutput=False)
    nb = nc.declare_dram_parameter("nb", [C], F32, isOutput=False)
    sel = nc.declare_dram_parameter("sel", [CT, 128, GROUPS], F32, isOutput=False)
    selb = nc.declare_dram_parameter("selb", [CT, GROUPS, 128], F32, isOutput=False)
    out = nc.declare_dram_parameter("out", [C, N], F32, isOutput=True)

    with tile.TileContext(nc) as tc, ExitStack() as ctx:
        singles = ctx.enter_context(tc.tile_pool(name="singles", bufs=1))
        small = ctx.enter_context(tc.tile_pool(name="small", bufs=4))
        work = ctx.enter_context(tc.tile_pool(name="work", bufs=2))
        expp = ctx.enter_context(tc.tile_pool(name="expp", bufs=4))
        gn_ctx = ExitStack()
        gnps = gn_ctx.enter_context(tc.tile_pool(name="gnps", bufs=5, space="PSUM"))

        x_sb = singles.tile([128, CT * N], F32)
        y_sb = singles.tile([128, CT * N], BF16)
        q_sb = singles.tile([128, 4 * N], BF16)
        k_sb = singles.tile([128, 4 * N], BF16)
        vplus = singles.tile([128, NT * HEADS * (D + 1)], BF16)  # [nt][h][65]
        av_sb = singles.tile([128, CT * N], BF16)
        wqkv_sb = singles.tile([128, CT * W3], BF16)
        wproj_sb = singles.tile([128, CT * C], BF16)
        bias_sb = singles.tile([128, 16], F32)  # 0:4 qb | 4:8 pbeff | 8:12 nw | 12:16 nb
        sel_sb = singles.tile([128, CT * GROUPS], F32)
        selb_sb = singles.tile([GROUPS, CT * 128], F32)
        zero_sb = singles.tile([128, 1], F32)
        eps_sb = singles.tile([128, 1], F32)
        ab_sb = singles.tile([128, 2 * CT], F32)  # a cols 0..3, b2 cols 4..7

        nc.vector.memset(zero_sb, 0.0)
        nc.vector.memset(eps_sb, EPS)
        nc.vector.memset(vplus, 1.0)

        # x/sel/bias first on the sync queue (groupnorm needs them immediately);
        # the 3.5MB of weights go on the gpsimd DMA queues in parallel.
        for t in range(CT):
            cs = slice(t * 128, (t + 1) * 128)
            nc.sync.dma_start(out=x_sb[:, t * N:(t + 1) * N], in_=x[cs, :])
        for t in range(CT):
            cs = slice(t * 128, (t + 1) * 128)
            nc.gpsimd.dma_start(out=wqkv_sb[:, t * W3:(t + 1) * W3], in_=qkvwT[cs, :])
            nc.gpsimd.dma_start(out=wproj_sb[:, t * C:(t + 1) * C], in_=projwT[cs, :])

        nc.sync.dma_start(
            out=sel_sb[:].rearrange("p (t g) -> p t g", g=GROUPS),
            in_=sel[:].rearrange("t p g -> p t g"),
        )
        nc.sync.dma_start(
            out=selb_sb[:].rearrange("g (t p) -> g t p", p=128),
            in_=selb[:].rearrange("t g p -> g t p"),
        )
        nc.sync.dma_start(out=bias_sb[:, 0:4], in_=qb[:].rearrange("(t p) -> p t", p=128))
        nc.sync.dma_start(out=bias_sb[:, 4:8], in_=pbeff[:].rearrange("(t p) -> p t", p=128))
        nc.sync.dma_start(out=bias_sb[:, 8:12], in_=nw[:].rearrange("(t p) -> p t", p=128))
        nc.sync.dma_start(out=bias_sb[:, 12:16], in_=nb[:].rearrange("(t p) -> p t", p=128))
        # Each TPB instruction has a single HW wait slot. Absorb the four bias
        # DMA semaphores onto the DVE clock early with tiny touch copies so
        # downstream DVE ops carry at most one (PSUM) wait.
        for j in range(4):
            bt = small.tile([1, 1], F32, tag="btouch", name=f"btouch{j}")
            nc.vector.tensor_copy(out=bt, in_=bias_sb[0:1, 4 * j:4 * j + 1])

        # bf16 staging copies of the selector matrices (entries are exact in
        # bf16); also collapses downstream matmul waits onto the DVE semaphore.
        selbf = singles.tile([128, CT * GROUPS], BF16)
        selbbf = singles.tile([GROUPS, CT * 128], BF16)
        nc.vector.tensor_copy(out=selbf, in_=sel_sb)
        nc.vector.tensor_copy(out=selbbf, in_=selb_sb)

        # ---------------- GroupNorm ----------------
        # Group aggregation uses hi/lo-split bf16 matmuls (exact selector,
        # f32 PSUM accumulation) to recover ~fp32 precision without the
        # fp32-matmul wait-slot limit.
        gps = gnps.tile([GROUPS, 2], F32, tag="gn")  # [E[x], E[x^2]] per group
        for t in range(CT):
            xt = x_sb[:, t * N:(t + 1) * N]
            st = small.tile([128, 2, 6], F32, tag="bn")
            nc.vector.bn_stats(out=st[:, 0, :], in_=xt[:, 0:512])
            nc.vector.bn_stats(out=st[:, 1, :], in_=xt[:, 512:1024])
            mv = small.tile([128, 2], F32, tag="mv")
            nc.vector.bn_aggr(out=mv, in_=st)
            mv2 = small.tile([128, 2], F32, tag="mv2")  # [mean, mean^2 + var]
            nc.vector.tensor_copy(out=mv2[:, 0:1], in_=mv[:, 0:1])
            nc.vector.tensor_scalar(
                out=mv2[:, 1:2], in0=mv[:, 0:1], scalar1=mv[:, 0:1],
                scalar2=mv[:, 1:2], op0=OP.mult, op1=OP.add,
            )
            mv2hi = small.tile([128, 2], BF16, tag="mv2hi")
            nc.vector.tensor_copy(out=mv2hi, in_=mv2)
            mv2lo = small.tile([128, 2], BF16, tag="mv2lo")
            nc.vector.tensor_tensor(out=mv2lo, in0=mv2, in1=mv2hi, op=OP.subtract)
            nc.tensor.matmul(
                gps, lhsT=selbf[:, t * GROUPS:(t + 1) * GROUPS], rhs=mv2hi,
                start=(t == 0), stop=False,
            )
            nc.tensor.matmul(
                gps, lhsT=selbf[:, t * GROUPS:(t + 1) * GROUPS], rhs=mv2lo,
                start=False, stop=(t == CT - 1),
            )
        m2g = small.tile([GROUPS, 1], F32, tag="m2g")
        nc.vector.tensor_scalar(
            out=m2g, in0=gps[:, 0:1], scalar1=gps[:, 0:1], scalar2=None, op0=OP.mult
        )
        vvar = small.tile([GROUPS, 1], F32, tag="vvar")
        nc.vector.tensor_tensor(out=vvar, in0=gps[:, 1:2], in1=m2g, op=OP.subtract)
        sq = small.tile([GROUPS, 1], F32, tag="sq")
        nc.scalar.activation(out=sq, in_=vvar, func=AF.Sqrt, bias=eps_sb[0:GROUPS], scale=1.0)
        gst = small.tile([GROUPS, 2], F32, tag="gst")  # [M, rstd]
        nc.vector.tensor_copy(out=gst[:, 0:1], in_=gps[:, 0:1])
        nc.vector.reciprocal(out=gst[:, 1:2], in_=sq)
        gsthi = small.tile([GROUPS, 2], BF16, tag="gsthi")
        nc.vector.tensor_copy(out=gsthi, in_=gst)
        gstlo = small.tile([GROUPS, 2], BF16, tag="gstlo")
        nc.vector.tensor_tensor(out=gstlo, in0=gst, in1=gsthi, op=OP.subtract)
        # PE toucher: absorb the DVE tick of gsthi/gstlo onto the PE clock so
        # the gbc matmuls carry a single wait (one HW wait slot per inst).
        nc.tensor.ldweights(weights=gstlo[0:1, 0:1])

        for t in range(CT):
            gbc = gnps.tile([128, 2], F32, tag="gn", name=f"gbc{t}")
            nc.tensor.matmul(
                gbc, lhsT=selbbf[0:GROUPS, t * 128:(t + 1) * 128], rhs=gsthi,
                start=True, stop=False,
            )
            nc.tensor.matmul(
                gbc, lhsT=selbbf[0:GROUPS, t * 128:(t + 1) * 128], rhs=gstlo,
                start=False, stop=True,
            )
            at = ab_sb[:, t:t + 1]
            b2t = ab_sb[:, CT + t:CT + t + 1]
            nc.vector.tensor_scalar(
                out=at, in0=bias_sb[:, 8 + t:9 + t], scalar1=gbc[:, 1:2],
                scalar2=None, op0=OP.mult,
            )
            mtmp = small.tile([128, 1], F32, tag="mtmp")
            nc.vector.tensor_scalar(
                out=mtmp, in0=at, scalar1=gbc[:, 0:1], scalar2=None, op0=OP.mult
            )
            nc.vector.tensor_tensor(
                out=b2t, in0=bias_sb[:, 12 + t:13 + t], in1=mtmp, op=OP.subtract
            )
            nc.vector.tensor_scalar(
                out=y_sb[:, t * N:(t + 1) * N], in0=x_sb[:, t * N:(t + 1) * N],
                scalar1=at, scalar2=b2t, op0=OP.mult, op1=OP.add,
            )

        gn_ctx.close()
        ps = ctx.enter_context(tc.tile_pool(name="ps", bufs=2, space="PSUM"))
        psav = ctx.enter_context(tc.tile_pool(name="psav", bufs=2, space="PSUM"))

        # ---------------- QKV ----------------
        # PE touchers: absorb the weight-DMA semaphores before the matmuls.
        for kt in range(CT):
            nc.tensor.ldweights(weights=wqkv_sb[0:1, kt * W3:kt * W3 + 1])
        # q/k in [row, n] layout: row-tiles 0..3 -> q, 4..7 -> k
        for mt in range(8):
            pp = ps.tile([128, N], F32, tag="ps")
            for nh in range(2):
                for kt in range(CT):
                    nc.tensor.matmul(
                        pp[:, nh * 512:(nh + 1) * 512],
                        lhsT=wqkv_sb[:, kt * W3 + mt * 128:kt * W3 + (mt + 1) * 128],
                        rhs=y_sb[:, kt * N + nh * 512:kt * N + (nh + 1) * 512],
                        start=(kt == 0), stop=(kt == CT - 1),
                    )
            if mt < 4:
                nc.vector.tensor_scalar(
                    out=q_sb[:, mt * N:(mt + 1) * N], in0=pp,
                    scalar1=bias_sb[:, mt:mt + 1], scalar2=None, op0=OP.add,
                )
            else:
                km = mt - 4
                nc.vector.tensor_copy(out=k_sb[:, km * N:(km + 1) * N], in_=pp)
        # v directly transposed: [n, vrow], interleaved with ones column per head
        for nt in range(NT):
            vp = ps.tile([128, 512], F32, tag="ps")
            for kt in range(CT):
                nc.tensor.matmul(
                    vp,
                    lhsT=y_sb[:, kt * N + nt * 128:kt * N + nt * 128 + 128],
                    rhs=wqkv_sb[:, kt * W3 + 2 * C:kt * W3 + 3 * C],
                    start=(kt == 0), stop=(kt == CT - 1),
                )
            dst = vplus[:, nt * HEADS * (D + 1):(nt + 1) * HEADS * (D + 1)]
            dst = dst.rearrange("p (h e) -> p h e", e=D + 1)[:, :, 0:D]
            nc.vector.tensor_copy(out=dst, in_=vp.rearrange("p (h e) -> p h e", e=D))

        # ---------------- Attention (per head pair) ----------------
        for pr in range(4):
            heads = ((2 * pr, 0), (2 * pr + 1, 64))
            etiles = {}
            for h, base in heads:
                etiles[h] = expp.tile([128, NT * N], BF16, tag="exp", name=f"exp{h}")
            for mt in range(NT):
                for h, base in heads:
                    sp = ps.tile([128, N], F32, tag="ps")
                    for nh in range(2):
                        nc.tensor.matmul(
                            sp[:, nh * 512:(nh + 1) * 512],
                            lhsT=k_sb[base:base + 64, pr * N + mt * 128:pr * N + mt * 128 + 128],
                            rhs=q_sb[base:base + 64, pr * N + nh * 512:pr * N + nh * 512 + 512],
                            start=True, stop=True,
                            tile_position=(base, 0),
                        )
                    nc.scalar.activation(
                        out=etiles[h][:, mt * N:(mt + 1) * N], in_=sp,
                        func=AF.Exp, bias=zero_sb, scale=1.0,
                    )
            if pr > 0:
                # Absorb the DVE tick of the previous pair's normalize ops so
                # the av matmuls (which reuse their PSUM slots) wait only on
                # the exp (ACT) semaphore.
                nc.tensor.ldweights(weights=av_sb[0:1, (pr - 1) * N:(pr - 1) * N + 1])
            # Z for both heads collected into a [128, 16] layout (Z[n] at
            # row n//8) so ONE wide reciprocal covers the pair at ~16
            # elems/lane instead of two [1, 1024] single-lane reciprocals.
            zp = small.tile([128, 16], F32, tag="zp", name=f"zp{pr}")
            apns = {}
            for h, base in heads:
                apn = psav.tile([D + 1, N], F32, tag="av", name=f"apn{h}")
                for mt in range(NT):
                    for nh in range(2):
                        nc.tensor.matmul(
                            apn[:, nh * 512:(nh + 1) * 512],
                            lhsT=vplus[:, mt * HEADS * (D + 1) + h * (D + 1):
                                       mt * HEADS * (D + 1) + (h + 1) * (D + 1)],
                            rhs=etiles[h][:, mt * N + nh * 512:mt * N + nh * 512 + 512],
                            start=(mt == 0), stop=(mt == NT - 1),
                        )
                apns[h] = apn
                zrow = small.tile([1, N], F32, tag="zrow", name=f"zrow{h}")
                nc.vector.tensor_copy(out=zrow, in_=apn[D:D + 1, :])
                nc.sync.dma_start(
                    out=zp[:, (h % 2) * 8:(h % 2) * 8 + 8],
                    in_=zrow.rearrange("o (p j) -> o p j", j=8),
                )
            rzp = small.tile([128, 16], F32, tag="rzp", name=f"rzp{pr}")
            nc.vector.reciprocal(out=rzp, in_=zp)
            for h, base in heads:
                rzrow = small.tile([1, N], F32, tag="rzrow", name=f"rzrow{h}")
                nc.sync.dma_start(
                    out=rzrow, in_=rzp[:, (h % 2) * 8:(h % 2) * 8 + 8]
                )
                rzb = work.tile([D, N], F32, tag="rzb")
                nc.gpsimd.partition_broadcast(out_ap=rzb, in_ap=rzrow)
                nc.vector.tensor_tensor(
                    out=av_sb[base:base + 64, pr * N:(pr + 1) * N],
                    in0=apns[h][0:D, :], in1=rzb, op=OP.mult,
                )

        # ---------------- Proj + residual ----------------
        for kt in range(CT):
            nc.tensor.ldweights(weights=wproj_sb[0:1, kt * C:kt * C + 1])
        for ct in range(CT):
            pp = ps.tile([128, N], F32, tag="ps")
            for nh in range(2):
                for kt in range(CT):
                    nc.tensor.matmul(
                        pp[:, nh * 512:(nh + 1) * 512],
                        lhsT=wproj_sb[:, kt * C + ct * 128:kt * C + (ct + 1) * 128],
                        rhs=av_sb[:, kt * N + nh * 512:kt * N + nh * 512 + 512],
                        start=(kt == 0), stop=(kt == CT - 1),
                    )
            ob = work.tile([128, N], F32, tag="osb")
            nc.vector.scalar_tensor_tensor(
                out=ob, in0=pp, scalar=bias_sb[:, 4 + ct:5 + ct],
                in1=x_sb[:, ct * N:(ct + 1) * N], op0=OP.add, op1=OP.add,
            )
            nc.sync.dma_start(out=out[ct * 128:(ct + 1) * 128, :], in_=ob)

    return nc


_CACHE = {}


def _get_nc():
    if "nc" not in _CACHE:
        nc = bacc.Bacc()
        _build(nc)
        nc.finalize()
        _CACHE["nc"] = nc
    return _CACHE["nc"]


def prepare_in_maps(x, norm_w, norm_b, qkv_w, qkv_b, proj_w, proj_b):
    x = np.asarray(x, np.float32)
    norm_w = np.asarray(norm_w, np.float32)
    norm_b = np.asarray(norm_b, np.float32)
    qkv_w = np.asarray(qkv_w, np.float32).copy()
    qkv_b = np.asarray(qkv_b, np.float32).copy()
    proj_w = np.asarray(proj_w, np.float32)
    proj_b = np.asarray(proj_b, np.float32)

    scale = D ** -0.5
    qkv_w[:C] *= scale
    qbias = (qkv_b[:C] * scale).astype(np.float32)
    vbias = qkv_b[2 * C:3 * C]
    qkvwT = np.ascontiguousarray(qkv_w.T).astype(ml_dtypes.bfloat16)
    projwT = np.ascontiguousarray(proj_w.T).astype(ml_dtypes.bfloat16)
    pb_eff = (proj_b + proj_w @ vbias).astype(np.float32)

    sel = np.zeros([CT, 128, GROUPS], np.float32)
    selb = np.zeros([CT, GROUPS, 128], np.float32)
    for t in range(CT):
        for p in range(128):
            g = (t * 128 + p) // GSIZE
            sel[t, p, g] = 1.0 / GSIZE
            selb[t, g, p] = 1.0
    shared = dict(
        qkvwT=qkvwT, projwT=projwT, qb=qbias, pbeff=pb_eff,
        nw=norm_w, nb=norm_b, sel=sel, selb=selb,
    )
    return [
        dict(x=np.ascontiguousarray(x[i].reshape(C, N)), **shared)
        for i in range(x.shape[0])
    ]


def run(in_maps, trace=False, **kwargs):
    return run_bass_kernel_spmd(
        _get_nc(), in_maps, core_ids=list(range(NCORES)), trace=trace, **kwargs
    )


def kernel(x, norm_w, norm_b, qkv_w, qkv_b, proj_w, proj_b):
    in_maps = prepare_in_maps(x, norm_w, norm_b, qkv_w, qkv_b, proj_w, proj_b)
    res = run(in_maps)
    b, c, h, w = np.asarray(x).shape
    return np.stack(
        [res.results[i]["out"].reshape(c, h, w) for i in range(b)]
    ).astype(np.float32)
